# revision 32
# baseline (speedup 1.0000x reference)
"""Trainium2 Bass kernel for nn_DependencyParser (BiLSTM + pairwise biaffine-ish scorer).

Sharding: data-parallel over batch B=16 across 8 cores (2 sentences/core);
embeddings + all weights replicated. Each core runs on-device:
  embedding gather (indirect DMA) -> 2-layer BiLSTM (gate-major recurrence with
  xp preloaded in PSUM, matmul-accumulate W_hh@h on top) -> pairwise scorer
  (a/b split of fc1, broadcast-add, tanh, fc2 contraction on PE).
Returns scores[b, j, i] per core; python assembles (L*L, B, 1) and adds fc2_b.
"""
import sys

if '/opt/trn_rl_repo' not in sys.path:
    sys.path.insert(0, '/opt/trn_rl_repo')

import numpy as np
import ml_dtypes

import concourse.bass as bass
import concourse.bacc as bacc
import concourse.mybir as mybir
import concourse.tile as tile
from concourse.bass_utils import run_bass_kernel_spmd

BF = np.float16
L = 128          # sequence length
B = 16           # batch
NCORES = 8
BPC = 2          # sentences per core
H = 128          # hidden per direction
WD = 100         # word emb dim
TD = 28          # tag emb dim
EMB = WD + TD    # 128
F32 = mybir.dt.float32
BF16 = mybir.dt.float16
I32 = mybir.dt.int32
SIG = mybir.ActivationFunctionType.Sigmoid
TANH = mybir.ActivationFunctionType.Tanh
IDENT = mybir.ActivationFunctionType.Identity
MUL = mybir.AluOpType.mult
ADD = mybir.AluOpType.add

_CACHE = {}
LAST_RESULTS = None  # BassKernelResults of the most recent run (for profiling)
TRACE = False


def _emit(nc, d):
    """Emit the full per-core program under a TileContext."""
    tc_ctx = tile.TileContext(nc)
    with tc_ctx as tc:
        with (
            tc.tile_pool(name="const", bufs=1) as cp,
            tc.tile_pool(name="work", bufs=2) as wp,
            tc.tile_pool(name="step", bufs=6) as sp,
            tc.tile_pool(name="psxp", bufs=1, space="PSUM") as pm,
            tc.tile_pool(name="psaux", bufs=2, space="PSUM") as pa,
        ):
            # ---- constants to SBUF ----
            whh_sb = cp.tile([128, 16, 128], BF16, tag="whh")
            nc.sync.dma_start(whh_sb[:], d['whh'][:].rearrange("l dd g k m -> k (l dd g) m"))
            wih0w_sb = cp.tile([WD, 8, 128], BF16, tag="wih0w")
            nc.sync.dma_start(wih0w_sb[:], d['wih0w'][:].rearrange("dd g k m -> k (dd g) m"))
            wih0t_sb = cp.tile([TD, 8, 128], BF16, tag="wih0t")
            nc.sync.dma_start(wih0t_sb[:], d['wih0t'][:].rearrange("dd g k m -> k (dd g) m"))
            wih1_sb = cp.tile([128, 16, 128], BF16, tag="wih1")
            nc.sync.dma_start(wih1_sb[:], d['wih1'][:].rearrange("dd g c k m -> k (dd g c) m"))
            bias_sb = cp.tile([1, 16, 128], BF16, tag="bias")
            nc.sync.dma_start(bias_sb[:], d['bias'][:].rearrange("l dd g o m -> o (l dd g) m"))
            wab_sb = cp.tile([128, 4, 100], BF16, tag="wab")
            nc.sync.dma_start(wab_sb[:], d['wab'][:].rearrange("s c k m -> k (s c) m"))
            fc1b_sb = cp.tile([100, 1], F32, tag="fc1b")
            nc.sync.dma_start(fc1b_sb[:], d['fc1b'][:])
            fc2t_sb = cp.tile([100, 1], BF16, tag="fc2t")
            nc.sync.dma_start(fc2t_sb[:], d['fc2t'][:])
            ident_sb = cp.tile([128, 128], F32, tag="ident")
            nc.sync.dma_start(ident_sb[:], d['ident'][:])
            widx_sb = cp.tile([128, 2], I32, tag="widx")
            nc.sync.dma_start(widx_sb[:], d['widx'][:].rearrange("c r o -> r (c o)"))
            pidx_sb = cp.tile([128, 2], I32, tag="pidx")
            nc.sync.dma_start(pidx_sb[:], d['pidx'][:].rearrange("c r o -> r (c o)"))

            ones_sb = cp.tile([1, 256], BF16, tag="ones")
            nc.vector.memset(ones_sb[:], 1.0)
            zeros_h = cp.tile([128, 4], BF16, tag="zeros")
            nc.vector.memset(zeros_h[:], 0.0)

            # ---- embedding gather + transpose -> xw_sb [100, 256], xt_sb [28, 256] ----
            xw_sb = cp.tile([WD, 256], BF16, tag="xw")
            xt_sb = cp.tile([TD, 256], BF16, tag="xt")
            for ch in range(2):
                wrows = wp.tile([128, WD], F32, tag="wrows")
                nc.gpsimd.indirect_dma_start(
                    out=wrows[:], out_offset=None, in_=d['word_emb'][:],
                    in_offset=bass.IndirectOffsetOnAxis(ap=widx_sb[:, ch:ch + 1], axis=0))
                et = pa.tile([128, 128], F32, tag="aux")
                nc.tensor.transpose(et[0:WD, :], wrows[:], ident_sb[:])
                nc.vector.tensor_copy(xw_sb[:, ch * 128:(ch + 1) * 128], et[0:WD, :])

                trows = wp.tile([128, TD], F32, tag="trows")
                nc.gpsimd.indirect_dma_start(
                    out=trows[:], out_offset=None, in_=d['tag_emb'][:],
                    in_offset=bass.IndirectOffsetOnAxis(ap=pidx_sb[:, ch:ch + 1], axis=0))
                et2 = pa.tile([128, 128], F32, tag="aux")
                nc.tensor.transpose(et2[0:TD, :], trows[:], ident_sb[:])
                nc.vector.tensor_copy(xt_sb[:, ch * 128:(ch + 1) * 128], et2[0:TD, :])

            # ---- two BiLSTM layers ----
            # layouts: xp PSUM [128, g, d, t, b] (bank g); h_hist [128, d, t, b]
            h_hists = []
            for p in range(2):
                h_hist = cp.tile([128, 2, 128, 2], BF16, tag=f"h{p}")
                xp = pm.tile([128, 4, 2, 128, 2], F32, tag="xp")
                # pre-pass: xp = W_ih @ x (+ bias), directly in PSUM
                for g in range(4):
                    for dd in range(2):
                        out = xp[:, g, dd, :, :]
                        if p == 0:
                            mms = [
                                (wih0w_sb[:, dd * 4 + g, :], xw_sb[:]),
                                (wih0t_sb[:, dd * 4 + g, :], xt_sb[:]),
                            ]
                        else:
                            mms = [
                                (wih1_sb[:, (dd * 4 + g) * 2 + kc, :],
                                 h_hists[0][:, kc, :, :])
                                for kc in range(2)
                            ]
                        first = (dd == 0)
                        for lhsT, rhs in mms:
                            nc.tensor.matmul(out, lhsT, rhs, start=first, stop=False,
                                             skip_group_check=True)
                            first = False
                        nc.tensor.matmul(out, bias_sb[0:1, (p * 2 + dd) * 4 + g, :],
                                         ones_sb[:], start=False, stop=False,
                                         skip_group_check=True)

                # recurrence
                c_t = cp.tile([128, 2, 2], F32, tag=f"c{p}")
                nc.vector.memset(c_t[:], 0.0)
                xp_ap = xp[:]
                h_ap = h_hist[:]
                for s in range(L):
                    # combined (dir f @ t=s, dir b @ t=L-1-s) strided view:
                    # col(g, j, b) = g*512 + j*(510-4s) + b, offset 2s
                    jstep = 510 - 4 * s
                    sig = sp.tile([128, 4, 2, 2], F32, tag="sig")
                    for dd in range(2):
                        t_in = s if dd == 0 else L - 1 - s
                        if s == 0:
                            h_prev = zeros_h[:, dd * 2:dd * 2 + 2]
                        elif dd == 0:
                            h_prev = h_hist[:, 0, s - 1, :]
                        else:
                            h_prev = h_hist[:, 1, L - s, :]
                        for g in range(4):
                            stop = (s == L - 1 and dd == 1)
                            nc.tensor.matmul(xp[:, g, dd, t_in, :],
                                             whh_sb[:, (p * 2 + dd) * 4 + g, :], h_prev,
                                             start=False, stop=stop, skip_group_check=True)
                    gate_src = bass.AP(xp_ap.tensor, xp_ap.offset + 2 * s,
                                       [xp_ap.ap[0], [512, 4], [jstep, 2], [1, 2]])
                    nc.scalar.activation(sig[:], gate_src, SIG)
                    tg = sp.tile([128, 2, 2], F32, tag="tg")
                    nc.vector.tensor_scalar(tg[:], sig[:, 2, :, :], 2.0, -1.0, MUL, ADD)
                    q = sp.tile([128, 2, 2], F32, tag="q")
                    nc.vector.tensor_tensor(q[:], sig[:, 1, :, :], c_t[:], MUL)
                    pp = sp.tile([128, 2, 2], F32, tag="pp")
                    nc.vector.tensor_tensor(pp[:], sig[:, 0, :, :], tg[:], MUL)
                    nc.vector.tensor_tensor(c_t[:], q[:], pp[:], ADD)
                    s2c = sp.tile([128, 2, 2], F32, tag="s2c")
                    nc.scalar.activation(s2c[:], c_t[:], SIG, scale=2.0)
                    tc_ = sp.tile([128, 2, 2], F32, tag="tc")
                    nc.vector.tensor_scalar(tc_[:], s2c[:], 2.0, -1.0, MUL, ADD)
                    h_dst = bass.AP(h_ap.tensor, h_ap.offset + 2 * s,
                                    [h_ap.ap[0], [jstep, 2], [1, 2]])
                    nc.vector.tensor_tensor(h_dst, sig[:, 3, :, :], tc_[:], MUL)
                h_hists.append(h_hist)

            # ---- pairwise scorer ----
            h1 = h_hists[1]
            a_sbs, bp_sbs = [], []
            for b in range(BPC):
                aps = pa.tile([128, 128], F32, tag="aux")
                for kc in range(2):
                    nc.tensor.matmul(aps[0:100, :], wab_sb[:, kc, :], h1[:, kc, :, b],
                                     start=(kc == 0), stop=(kc == 1))
                a_sb = cp.tile([100, 128], F32, tag=f"a{b}")
                nc.vector.tensor_copy(a_sb[:], aps[0:100, :])
                a_sbs.append(a_sb)
                bps = pa.tile([128, 128], F32, tag="aux")
                for kc in range(2):
                    nc.tensor.matmul(bps[0:100, :], wab_sb[:, 2 + kc, :], h1[:, kc, :, b],
                                     start=(kc == 0), stop=(kc == 1))
                bp_sb = cp.tile([100, 128], BF16, tag=f"bp{b}")
                nc.scalar.activation(bp_sb[:], bps[0:100, :], IDENT, bias=fc1b_sb[:])
                bp_sbs.append(bp_sb)

            for b in range(BPC):
                scps = pa.tile([128, 128], F32, tag="aux")
                bp_ap = bp_sbs[b][:]
                a_ap = a_sbs[b][:]
                IC = 16  # i-chunk size
                for ic in range(128 // IC):
                    ha = wp.tile([100, IC, 128], BF16, tag="ha")
                    # hid_arg[k, i, j] = bp[k, j] + a[k, ic*IC+i] via broadcast APs
                    bp_b = bass.AP(bp_ap.tensor, bp_ap.offset,
                                   [bp_ap.ap[0], [0, IC], [1, 128]])
                    a_b = bass.AP(a_ap.tensor, a_ap.offset + ic * IC,
                                  [a_ap.ap[0], [1, IC], [0, 128]])
                    nc.vector.tensor_tensor(ha[:], bp_b, a_b, ADD)
                    th = wp.tile([100, IC, 128], BF16, tag="th")
                    nc.scalar.activation(th[:], ha[:], TANH)
                    for il in range(IC):
                        i = ic * IC + il
                        nc.tensor.matmul(scps[:, i:i + 1], th[:, il, :], fc2t_sb[:],
                                         start=True, stop=True)
                sco = wp.tile([128, 128], F32, tag="sco")
                nc.vector.tensor_copy(sco[:], scps[:])
                nc.sync.dma_start(d['out'][b, :, :], sco[:])


def _build():
    if 'nc' in _CACHE:
        return _CACHE['nc']
    nc = bacc.Bacc("TRN2", target_bir_lowering=False, debug=False)
    d = {
        'widx': nc.dram_tensor("widx", [2, 128, 1], I32, kind="ExternalInput"),
        'pidx': nc.dram_tensor("pidx", [2, 128, 1], I32, kind="ExternalInput"),
        'word_emb': nc.dram_tensor("word_emb", [50000, WD], F32, kind="ExternalInput"),
        'tag_emb': nc.dram_tensor("tag_emb", [50, TD], F32, kind="ExternalInput"),
        'wih0w': nc.dram_tensor("wih0w", [2, 4, WD, 128], BF16, kind="ExternalInput"),
        'wih0t': nc.dram_tensor("wih0t", [2, 4, TD, 128], BF16, kind="ExternalInput"),
        'wih1': nc.dram_tensor("wih1", [2, 4, 2, 128, 128], BF16, kind="ExternalInput"),
        'whh': nc.dram_tensor("whh", [2, 2, 4, 128, 128], BF16, kind="ExternalInput"),
        'bias': nc.dram_tensor("bias", [2, 2, 4, 1, 128], BF16, kind="ExternalInput"),
        'wab': nc.dram_tensor("wab", [2, 2, 128, 100], BF16, kind="ExternalInput"),
        'fc1b': nc.dram_tensor("fc1b", [100, 1], F32, kind="ExternalInput"),
        'fc2t': nc.dram_tensor("fc2t", [100, 1], BF16, kind="ExternalInput"),
        'ident': nc.dram_tensor("ident", [128, 128], F32, kind="ExternalInput"),
        'out': nc.dram_tensor("out", [BPC, 128, 128], F32, kind="ExternalOutput"),
    }
    _emit(nc, d)
    nc.compile()
    _CACHE['nc'] = nc
    return nc


def _prep_weights(inputs):
    """Shared (replicated) weight arrays, transformed for the kernel layout."""
    wih0w = np.zeros((2, 4, WD, 128), BF)
    wih0t = np.zeros((2, 4, TD, 128), BF)
    wih1 = np.zeros((2, 4, 2, 128, 128), BF)
    whh = np.zeros((2, 2, 4, 128, 128), BF)
    bias = np.zeros((2, 2, 4, 1, 128), BF)
    for l in range(2):
        for dd, dn in enumerate('fb'):
            wi = np.asarray(inputs[f'w_ih_l{l}{dn}'], np.float32).copy()
            wh = np.asarray(inputs[f'w_hh_l{l}{dn}'], np.float32).copy()
            bb = (np.asarray(inputs[f'b_ih_l{l}{dn}'], np.float32)
                  + np.asarray(inputs[f'b_hh_l{l}{dn}'], np.float32)).copy()
            # scale the cell-candidate ('g') gate by 2 for the 2*sigmoid(2x)-1 tanh trick
            wi[2 * H:3 * H] *= 2.0
            wh[2 * H:3 * H] *= 2.0
            bb[2 * H:3 * H] *= 2.0
            for g in range(4):
                gs = slice(g * H, (g + 1) * H)
                whh[l, dd, g] = wh[gs, :].T.astype(BF)
                bias[l, dd, g, 0] = bb[gs].astype(BF)
                if l == 0:
                    wih0w[dd, g] = wi[gs, 0:WD].T.astype(BF)
                    wih0t[dd, g] = wi[gs, WD:128].T.astype(BF)
                else:
                    for kc in range(2):
                        wih1[dd, g, kc] = wi[gs, kc * 128:(kc + 1) * 128].T.astype(BF)
    fc1_w = np.asarray(inputs['fc1_w'], np.float32)
    wab = np.zeros((2, 2, 128, 100), BF)
    for s in range(2):
        for kc in range(2):
            wab[s, kc] = fc1_w[:, s * 256 + kc * 128: s * 256 + (kc + 1) * 128].T.astype(BF)
    return {
        'word_emb': np.ascontiguousarray(np.asarray(inputs['word_emb'], np.float32)),
        'tag_emb': np.ascontiguousarray(np.asarray(inputs['tag_emb'], np.float32)),
        'wih0w': wih0w, 'wih0t': wih0t, 'wih1': wih1, 'whh': whh, 'bias': bias, 'wab': wab,
        'fc1b': np.asarray(inputs['fc1_b'], np.float32).reshape(100, 1).copy(),
        'fc2t': np.asarray(inputs['fc2_w'], np.float32).reshape(1, 100).T.astype(BF).copy(),
        'ident': np.eye(128, dtype=np.float32),
    }


def make_in_maps(inputs):
    shared = _prep_weights(inputs)
    widx = np.asarray(inputs['words_idx']).astype(np.int32)  # [16, 128]
    pidx = np.asarray(inputs['pos_idx']).astype(np.int32)
    in_maps = []
    for c in range(NCORES):
        # x column order (t, b) interleaved: col n = t*2 + b_local, chunked 2x128
        w = widx[BPC * c: BPC * (c + 1)].T.reshape(2, 128, 1).copy()
        p = pidx[BPC * c: BPC * (c + 1)].T.reshape(2, 128, 1).copy()
        m = dict(shared)
        m['widx'] = w
        m['pidx'] = p
        in_maps.append(m)
    return in_maps


def kernel(**inputs):
    global LAST_RESULTS
    nc = _build()
    in_maps = make_in_maps(inputs)
    res = run_bass_kernel_spmd(nc, in_maps, list(range(NCORES)), trace=TRACE)
    LAST_RESULTS = res
    outs = [r['out'] for r in res.results]          # each [2, 128(j), 128(i)]
    arr = np.concatenate(outs, axis=0)              # [16, j, i]
    fin = arr.transpose(2, 1, 0).reshape(L * L, B, 1)  # [(i,j), b, 1]
    fin = fin + np.asarray(inputs['fc2_b'], np.float32).reshape(1, 1, 1)
    return fin.astype(np.float32)


# revision 43
# speedup vs baseline: 191.6349x; 191.6349x over previous
"""Trainium2 Bass kernel for nn_DependencyParser (BiLSTM + pairwise biaffine-ish scorer).

Sharding: data-parallel over batch B=16 across 8 cores (2 sentences/core);
embeddings + all weights replicated. Each core runs on-device:
  embedding gather (indirect DMA) -> 2-layer BiLSTM (gate-major recurrence with
  xp preloaded in PSUM, matmul-accumulate W_hh@h on top) -> pairwise scorer
  (a/b split of fc1, broadcast-add, tanh, fc2 contraction on PE).
Returns scores[b, j, i] per core; python assembles (L*L, B, 1) and adds fc2_b.
"""
import sys

if '/opt/trn_rl_repo' not in sys.path:
    sys.path.insert(0, '/opt/trn_rl_repo')

import numpy as np
import ml_dtypes

import concourse.bass as bass
import concourse.bacc as bacc
import concourse.mybir as mybir
import concourse.tile as tile
from concourse.bass_utils import run_bass_kernel_spmd

BF = np.float16
L = 128          # sequence length
B = 16           # batch
NCORES = 8
BPC = 2          # sentences per core
H = 128          # hidden per direction
WD = 100         # word emb dim
TD = 28          # tag emb dim
EMB = WD + TD    # 128
F32 = mybir.dt.float32
BF16 = mybir.dt.float16
I32 = mybir.dt.int32
SIG = mybir.ActivationFunctionType.Sigmoid
TANH = mybir.ActivationFunctionType.Tanh
IDENT = mybir.ActivationFunctionType.Identity
MUL = mybir.AluOpType.mult
ADD = mybir.AluOpType.add

_CACHE = {}
LAST_RESULTS = None  # BassKernelResults of the most recent run (for profiling)
TRACE = False


def _emit(nc, d):
    """Emit the full per-core program under a TileContext."""
    tc_ctx = tile.TileContext(nc)
    with tc_ctx as tc:
        with (
            tc.tile_pool(name="const", bufs=1) as cp,
            tc.tile_pool(name="work", bufs=3) as wp,
            tc.tile_pool(name="step", bufs=6) as sp,
            tc.tile_pool(name="psxp", bufs=1, space="PSUM") as pm,
            tc.tile_pool(name="psaux", bufs=2, space="PSUM") as pa,
        ):
            # ---- constants to SBUF (small index/identity tensors first: they
            # gate the embedding gathers; big weight DMAs stream behind) ----
            widx_sb = cp.tile([128, 2], I32, tag="widx")
            nc.sync.dma_start(widx_sb[:], d['widx'][:].rearrange("c r o -> r (c o)"))
            pidx_sb = cp.tile([128, 2], I32, tag="pidx")
            nc.sync.dma_start(pidx_sb[:], d['pidx'][:].rearrange("c r o -> r (c o)"))
            ident_sb = cp.tile([128, 128], F32, tag="ident")
            nc.sync.dma_start(ident_sb[:], d['ident'][:])
            whh_sb = cp.tile([128, 16, 128], BF16, tag="whh")
            nc.sync.dma_start(whh_sb[:], d['whh'][:].rearrange("l dd g k m -> k (l dd g) m"))
            wih0w_sb = cp.tile([WD, 8, 128], BF16, tag="wih0w")
            nc.sync.dma_start(wih0w_sb[:], d['wih0w'][:].rearrange("dd g k m -> k (dd g) m"))
            wih0t_sb = cp.tile([TD, 8, 128], BF16, tag="wih0t")
            nc.sync.dma_start(wih0t_sb[:], d['wih0t'][:].rearrange("dd g k m -> k (dd g) m"))
            wih1_sb = cp.tile([128, 16, 128], BF16, tag="wih1")
            nc.sync.dma_start(wih1_sb[:], d['wih1'][:].rearrange("dd g c k m -> k (dd g c) m"))
            bias_sb = cp.tile([1, 16, 128], BF16, tag="bias")
            nc.sync.dma_start(bias_sb[:], d['bias'][:].rearrange("l dd g o m -> o (l dd g) m"))
            wab_sb = cp.tile([128, 4, 100], BF16, tag="wab")
            nc.sync.dma_start(wab_sb[:], d['wab'][:].rearrange("s c k m -> k (s c) m"))
            fc1b_sb = cp.tile([100, 1], F32, tag="fc1b")
            nc.sync.dma_start(fc1b_sb[:], d['fc1b'][:])
            fc2t_sb = cp.tile([100, 1], BF16, tag="fc2t")
            nc.sync.dma_start(fc2t_sb[:], d['fc2t'][:])

            ones_sb = cp.tile([1, 256], BF16, tag="ones")
            nc.vector.memset(ones_sb[:], 1.0)
            zeros_h = cp.tile([128, 4], BF16, tag="zeros")
            nc.vector.memset(zeros_h[:], 0.0)

            # ---- embedding gather + transpose -> xw_sb [100, 256], xt_sb [28, 256] ----
            xw_sb = cp.tile([WD, 256], BF16, tag="xw")
            xt_sb = cp.tile([TD, 256], BF16, tag="xt")
            for ch in range(2):
                wrows = wp.tile([128, WD], F32, tag="wrows")
                nc.gpsimd.indirect_dma_start(
                    out=wrows[:], out_offset=None, in_=d['word_emb'][:],
                    in_offset=bass.IndirectOffsetOnAxis(ap=widx_sb[:, ch:ch + 1], axis=0))
                et = pa.tile([128, 128], F32, tag="aux")
                nc.tensor.transpose(et[0:WD, :], wrows[:], ident_sb[:])
                nc.vector.tensor_copy(xw_sb[:, ch * 128:(ch + 1) * 128], et[0:WD, :])

                trows = wp.tile([128, TD], F32, tag="trows")
                nc.gpsimd.indirect_dma_start(
                    out=trows[:], out_offset=None, in_=d['tag_emb'][:],
                    in_offset=bass.IndirectOffsetOnAxis(ap=pidx_sb[:, ch:ch + 1], axis=0))
                et2 = pa.tile([128, 128], F32, tag="aux")
                nc.tensor.transpose(et2[0:TD, :], trows[:], ident_sb[:])
                nc.vector.tensor_copy(xt_sb[:, ch * 128:(ch + 1) * 128], et2[0:TD, :])

            # ---- two BiLSTM layers ----
            # layouts: xp PSUM [128, g, d, t, b] (bank g); h_hist [128, d, t, b]
            h_hists = []
            for p in range(2):
                h_hist = cp.tile([128, 2, 128, 2], BF16, tag=f"h{p}")
                xp = pm.tile([128, 4, 2, 128, 2], F32, tag="xp")
                # pre-pass: xp = W_ih @ x (+ bias), directly in PSUM
                for g in range(4):
                    for dd in range(2):
                        out = xp[:, g, dd, :, :]
                        if p == 0:
                            mms = [
                                (wih0w_sb[:, dd * 4 + g, :], xw_sb[:]),
                                (wih0t_sb[:, dd * 4 + g, :], xt_sb[:]),
                            ]
                        else:
                            mms = [
                                (wih1_sb[:, (dd * 4 + g) * 2 + kc, :],
                                 h_hists[0][:, kc, :, :])
                                for kc in range(2)
                            ]
                        first = (dd == 0)
                        for lhsT, rhs in mms:
                            nc.tensor.matmul(out, lhsT, rhs, start=first, stop=False,
                                             skip_group_check=True)
                            first = False
                        nc.tensor.matmul(out, bias_sb[0:1, (p * 2 + dd) * 4 + g, :],
                                         ones_sb[:], start=False, stop=False,
                                         skip_group_check=True)

                # recurrence
                c_t = cp.tile([128, 2, 2], F32, tag=f"c{p}")
                nc.vector.memset(c_t[:], 0.0)
                xp_ap = xp[:]
                h_ap = h_hist[:]
                for s in range(L):
                    # combined (dir f @ t=s, dir b @ t=L-1-s) strided view:
                    # col(g, j, b) = g*512 + j*(510-4s) + b, offset 2s
                    jstep = 510 - 4 * s
                    sig = sp.tile([128, 4, 2, 2], F32, tag="sig")
                    for dd in range(2):
                        t_in = s if dd == 0 else L - 1 - s
                        if s == 0:
                            h_prev = zeros_h[:, dd * 2:dd * 2 + 2]
                        elif dd == 0:
                            h_prev = h_hist[:, 0, s - 1, :]
                        else:
                            h_prev = h_hist[:, 1, L - s, :]
                        for g in range(4):
                            stop = (s == L - 1 and dd == 1)
                            nc.tensor.matmul(xp[:, g, dd, t_in, :],
                                             whh_sb[:, (p * 2 + dd) * 4 + g, :], h_prev,
                                             start=False, stop=stop, skip_group_check=True)
                    gate_src = bass.AP(xp_ap.tensor, xp_ap.offset + 2 * s,
                                       [xp_ap.ap[0], [512, 4], [jstep, 2], [1, 2]])
                    nc.scalar.activation(sig[:], gate_src, SIG)
                    tg = sp.tile([128, 2, 2], F32, tag="tg")
                    nc.vector.tensor_scalar(tg[:], sig[:, 2, :, :], 2.0, -1.0, MUL, ADD)
                    q = sp.tile([128, 2, 2], F32, tag="q")
                    nc.vector.tensor_tensor(q[:], sig[:, 1, :, :], c_t[:], MUL)
                    pp = sp.tile([128, 2, 2], F32, tag="pp")
                    nc.vector.tensor_tensor(pp[:], sig[:, 0, :, :], tg[:], MUL)
                    nc.vector.tensor_tensor(c_t[:], q[:], pp[:], ADD)
                    s2c = sp.tile([128, 2, 2], F32, tag="s2c")
                    nc.scalar.activation(s2c[:], c_t[:], SIG, scale=2.0)
                    tc_ = sp.tile([128, 2, 2], F32, tag="tc")
                    nc.vector.tensor_scalar(tc_[:], s2c[:], 2.0, -1.0, MUL, ADD)
                    h_dst = bass.AP(h_ap.tensor, h_ap.offset + 2 * s,
                                    [h_ap.ap[0], [jstep, 2], [1, 2]])
                    nc.vector.tensor_tensor(h_dst, sig[:, 3, :, :], tc_[:], MUL)
                h_hists.append(h_hist)

            # ---- pairwise scorer ----
            h1 = h_hists[1]
            a_sbs, bp_sbs = [], []
            for b in range(BPC):
                aps = pa.tile([128, 128], F32, tag="aux")
                for kc in range(2):
                    nc.tensor.matmul(aps[0:100, :], wab_sb[:, kc, :], h1[:, kc, :, b],
                                     start=(kc == 0), stop=(kc == 1))
                a_sb = cp.tile([100, 128], F32, tag=f"a{b}")
                nc.vector.tensor_copy(a_sb[:], aps[0:100, :])
                a_sbs.append(a_sb)
                bps = pa.tile([128, 128], F32, tag="aux")
                for kc in range(2):
                    nc.tensor.matmul(bps[0:100, :], wab_sb[:, 2 + kc, :], h1[:, kc, :, b],
                                     start=(kc == 0), stop=(kc == 1))
                bp_sb = cp.tile([100, 128], BF16, tag=f"bp{b}")
                nc.scalar.activation(bp_sb[:], bps[0:100, :], IDENT, bias=fc1b_sb[:])
                bp_sbs.append(bp_sb)

            for b in range(BPC):
                scps = pa.tile([128, 128], F32, tag="aux")
                bp_ap = bp_sbs[b][:]
                a_ap = a_sbs[b][:]
                IC = 16  # i-chunk size
                for ic in range(128 // IC):
                    ha = wp.tile([100, IC, 128], BF16, tag="ha")
                    # hid_arg[k, i, j] = bp[k, j] + a[k, ic*IC+i] via broadcast APs
                    bp_b = bass.AP(bp_ap.tensor, bp_ap.offset,
                                   [bp_ap.ap[0], [0, IC], [1, 128]])
                    a_b = bass.AP(a_ap.tensor, a_ap.offset + ic * IC,
                                  [a_ap.ap[0], [1, IC], [0, 128]])
                    # alternate the broadcast-add between DVE and GPSIMD
                    eng = nc.vector if ic % 3 != 2 else nc.gpsimd
                    eng.tensor_tensor(ha[:], bp_b, a_b, ADD)
                    th = wp.tile([100, IC, 128], BF16, tag="th")
                    nc.scalar.activation(th[:], ha[:], TANH)
                    for il in range(IC):
                        i = ic * IC + il
                        nc.tensor.matmul(scps[:, i:i + 1], th[:, il, :], fc2t_sb[:],
                                         start=True, stop=True)
                sco = wp.tile([128, 128], F32, tag="sco")
                nc.vector.tensor_copy(sco[:], scps[:])
                nc.sync.dma_start(d['out'][b, :, :], sco[:])


def _build():
    if 'nc' in _CACHE:
        return _CACHE['nc']
    nc = bacc.Bacc("TRN2", target_bir_lowering=False, debug=False)
    d = {
        'widx': nc.dram_tensor("widx", [2, 128, 1], I32, kind="ExternalInput"),
        'pidx': nc.dram_tensor("pidx", [2, 128, 1], I32, kind="ExternalInput"),
        'word_emb': nc.dram_tensor("word_emb", [50000, WD], F32, kind="ExternalInput"),
        'tag_emb': nc.dram_tensor("tag_emb", [50, TD], F32, kind="ExternalInput"),
        'wih0w': nc.dram_tensor("wih0w", [2, 4, WD, 128], BF16, kind="ExternalInput"),
        'wih0t': nc.dram_tensor("wih0t", [2, 4, TD, 128], BF16, kind="ExternalInput"),
        'wih1': nc.dram_tensor("wih1", [2, 4, 2, 128, 128], BF16, kind="ExternalInput"),
        'whh': nc.dram_tensor("whh", [2, 2, 4, 128, 128], BF16, kind="ExternalInput"),
        'bias': nc.dram_tensor("bias", [2, 2, 4, 1, 128], BF16, kind="ExternalInput"),
        'wab': nc.dram_tensor("wab", [2, 2, 128, 100], BF16, kind="ExternalInput"),
        'fc1b': nc.dram_tensor("fc1b", [100, 1], F32, kind="ExternalInput"),
        'fc2t': nc.dram_tensor("fc2t", [100, 1], BF16, kind="ExternalInput"),
        'ident': nc.dram_tensor("ident", [128, 128], F32, kind="ExternalInput"),
        'out': nc.dram_tensor("out", [BPC, 128, 128], F32, kind="ExternalOutput"),
    }
    _emit(nc, d)
    nc.compile()
    _CACHE['nc'] = nc
    return nc


def _prep_weights(inputs):
    """Shared (replicated) weight arrays, transformed for the kernel layout."""
    wih0w = np.zeros((2, 4, WD, 128), BF)
    wih0t = np.zeros((2, 4, TD, 128), BF)
    wih1 = np.zeros((2, 4, 2, 128, 128), BF)
    whh = np.zeros((2, 2, 4, 128, 128), BF)
    bias = np.zeros((2, 2, 4, 1, 128), BF)
    for l in range(2):
        for dd, dn in enumerate('fb'):
            wi = np.asarray(inputs[f'w_ih_l{l}{dn}'], np.float32).copy()
            wh = np.asarray(inputs[f'w_hh_l{l}{dn}'], np.float32).copy()
            bb = (np.asarray(inputs[f'b_ih_l{l}{dn}'], np.float32)
                  + np.asarray(inputs[f'b_hh_l{l}{dn}'], np.float32)).copy()
            # scale the cell-candidate ('g') gate by 2 for the 2*sigmoid(2x)-1 tanh trick
            wi[2 * H:3 * H] *= 2.0
            wh[2 * H:3 * H] *= 2.0
            bb[2 * H:3 * H] *= 2.0
            for g in range(4):
                gs = slice(g * H, (g + 1) * H)
                whh[l, dd, g] = wh[gs, :].T.astype(BF)
                bias[l, dd, g, 0] = bb[gs].astype(BF)
                if l == 0:
                    wih0w[dd, g] = wi[gs, 0:WD].T.astype(BF)
                    wih0t[dd, g] = wi[gs, WD:128].T.astype(BF)
                else:
                    for kc in range(2):
                        wih1[dd, g, kc] = wi[gs, kc * 128:(kc + 1) * 128].T.astype(BF)
    fc1_w = np.asarray(inputs['fc1_w'], np.float32)
    wab = np.zeros((2, 2, 128, 100), BF)
    for s in range(2):
        for kc in range(2):
            wab[s, kc] = fc1_w[:, s * 256 + kc * 128: s * 256 + (kc + 1) * 128].T.astype(BF)
    return {
        'word_emb': np.ascontiguousarray(np.asarray(inputs['word_emb'], np.float32)),
        'tag_emb': np.ascontiguousarray(np.asarray(inputs['tag_emb'], np.float32)),
        'wih0w': wih0w, 'wih0t': wih0t, 'wih1': wih1, 'whh': whh, 'bias': bias, 'wab': wab,
        'fc1b': np.asarray(inputs['fc1_b'], np.float32).reshape(100, 1).copy(),
        'fc2t': np.asarray(inputs['fc2_w'], np.float32).reshape(1, 100).T.astype(BF).copy(),
        'ident': np.eye(128, dtype=np.float32),
    }


def make_in_maps(inputs):
    shared = _prep_weights(inputs)
    widx = np.asarray(inputs['words_idx']).astype(np.int32)  # [16, 128]
    pidx = np.asarray(inputs['pos_idx']).astype(np.int32)
    in_maps = []
    for c in range(NCORES):
        # x column order (t, b) interleaved: col n = t*2 + b_local, chunked 2x128
        w = widx[BPC * c: BPC * (c + 1)].T.reshape(2, 128, 1).copy()
        p = pidx[BPC * c: BPC * (c + 1)].T.reshape(2, 128, 1).copy()
        m = dict(shared)
        m['widx'] = w
        m['pidx'] = p
        in_maps.append(m)
    return in_maps


def kernel(**inputs):
    global LAST_RESULTS
    nc = _build()
    in_maps = make_in_maps(inputs)
    res = run_bass_kernel_spmd(nc, in_maps, list(range(NCORES)), trace=TRACE)
    LAST_RESULTS = res
    outs = [r['out'] for r in res.results]          # each [2, 128(j), 128(i)]
    arr = np.concatenate(outs, axis=0)              # [16, j, i]
    fin = arr.transpose(2, 1, 0).reshape(L * L, B, 1)  # [(i,j), b, 1]
    fin = fin + np.asarray(inputs['fc2_b'], np.float32).reshape(1, 1, 1)
    return fin.astype(np.float32)
